# revision 36
# baseline (speedup 1.0000x reference)
"""Correlation kernel (FlowNet-style, W-displacement only) for Trainium2.

out[b, j, h, w] = mean_c f1[b,c,h,w] * f2pad[b,c,h,w+j],  j in [0, 81), pad=40.

Sharding: data-parallel over batch B=8 across 8 cores (1 batch elem/core).

Device-side work per core (per h row):
  1. Convert f1/f2 chunks fp32 -> fp16 (plain contiguous copies, ACT/DVE).
  2. 3 fp16 matmuls (contraction over C=128 on partitions) produce Gram tiles
     G[w', u] = sum_c f1[c, w0+w'] * f2[c, u0+u] in PSUM (fp32).
  3. ACT/DVE/Pool copy Gram cols PSUM -> SBUF fp16, packed 480 cols/row.
  4. One chunked DMA per 8 rows dumps the packed Gram tiles to DRAM (fp16).

No f2 zero-padding on device: each block's rhs window is clamped to the valid
[0, W) range and the host zero-pads the out-of-range displacements. The
diagonal band extraction (out[j,w] = G[w, w+j-40]) is a shear, which no
on-chip engine can address (per-partition offsets are illegal); the host does
it for free with numpy as_strided during the unshard step. Per-core DRAM
traffic is ~43MB (31.5MB in + 11.8MB out) vs ~84MB for a bounce-based kernel.

DMA queues: input loads ride the SP (sync) ring, dumps ride the ACT (scalar)
ring, so a dump waiting on compute never head-of-line blocks the next chunk's
loads.
"""

import numpy as np
from contextlib import ExitStack

B, C, H, W = 8, 128, 96, 320
D = 40
J = 2 * D + 1  # 81
N_CORES = 8

HCHUNK = 8
NCHUNK = H // HCHUNK
WB = [0, 128, 256]     # w-block starts (lhsT = f1 cols [w0, w0+M))
WN = [128, 128, 128]   # lhsT widths (block 2 spans 64 slack cols)
US = [0, 88, 216]      # rhs window starts (clamped to [0, W))
CPB = [168, 208, 104]  # Gram cols per block (= clamped band cover)
COFF = [0, 168, 376]   # col offsets in the packed dump row
DUMPW = 480
SLACK = 64             # f1h slack so block-2 lhsT can be 128 wide


def _build():
    from concourse import bacc, mybir
    import concourse.tile as tile

    f32 = mybir.dt.float32
    f16 = mybir.dt.float16
    nc = bacc.Bacc(
        "TRN2",
        target_bir_lowering=False,
        debug=False,
        enable_asserts=False,
        num_devices=N_CORES,
    )
    f1 = nc.dram_tensor("f1", [C, H, W], f32, kind="ExternalInput").ap()
    f2 = nc.dram_tensor("f2", [C, H, W], f32, kind="ExternalInput").ap()
    outa = nc.dram_tensor("outa", [128, H, COFF[2]], f16, kind="ExternalOutput").ap()
    outb = nc.dram_tensor("outb", [64, H, CPB[2]], f16, kind="ExternalOutput").ap()

    with tile.TileContext(nc) as tc, ExitStack() as ctx:
        f1r_pool = ctx.enter_context(tc.tile_pool(name="f1r", bufs=3))
        f2r_pool = ctx.enter_context(tc.tile_pool(name="f2r", bufs=3))
        f1h_pool = ctx.enter_context(tc.tile_pool(name="f1h", bufs=3))
        f2h_pool = ctx.enter_context(tc.tile_pool(name="f2h", bufs=3))
        g_pool = ctx.enter_context(tc.tile_pool(name="gsb", bufs=3))
        gb_pool = ctx.enter_context(tc.tile_pool(name="gbsb", bufs=3))
        ps01_pool = ctx.enter_context(tc.tile_pool(name="ps01", bufs=5, space="PSUM"))
        ps2_pool = ctx.enter_context(tc.tile_pool(name="ps2", bufs=3, space="PSUM"))

        chunks = [(i * HCHUNK, HCHUNK) for i in range(NCHUNK - 1)]
        chunks += [((NCHUNK - 1) * HCHUNK, 4), ((NCHUNK - 1) * HCHUNK + 4, 4)]

        def emit_load_conv(ci, h0, hc):
            """Loads + fp16 conversions for one chunk; returns (f1s, f2s)."""
            edge = ci < 2 or ci >= len(chunks) - 3
            f1r = f1r_pool.tile([C, hc * W], f32)
            nc.sync.dma_start(f1r[:], f1[:, h0 : h0 + hc, :])
            f2r = f2r_pool.tile([C, hc * W], f32)
            nc.sync.dma_start(f2r[:], f2[:, h0 : h0 + hc, :])
            nh = hc * W
            f1s = f1h_pool.tile([C, nh + SLACK], f16)
            nc.scalar.copy(f1s[:, 0:nh], f1r[:, 0:nh])
            nc.gpsimd.memset(f1s[:, nh:], 0.0)
            f2s = f2h_pool.tile([C, nh], f16)
            if edge:
                na = (nh * 9) // 20
                nc.scalar.copy(f2s[:, 0:na], f2r[:, 0:na])
                nc.vector.tensor_copy(f2s[:, na:nh], f2r[:, na:nh])
            else:
                na, nd = nh // 5, nh // 2
                nc.scalar.copy(f2s[:, 0:na], f2r[:, 0:na])
                nc.vector.tensor_copy(f2s[:, na:nd], f2r[:, na:nd])
                nc.gpsimd.tensor_copy(f2s[:, nd:nh], f2r[:, nd:nh])
            return f1s, f2s

        def emit_compute(h0, hc, f1s, f2s):
            """Matmuls, PSUM->fp16 staging, and dumps for one chunk."""
            ga = g_pool.tile([C, hc * COFF[2]], f16, tag="ga")
            gb = gb_pool.tile([64, hc * CPB[2]], f16, tag="gb")
            for h4 in range(0, hc, 4):
                # block-2 Gram tiles for 4 rows share one PSUM bank
                p2 = ps2_pool.tile([128, 4 * CPB[2]], f32, tag="p2")
                for dh in range(4):
                    h = h4 + dh
                    base = h * W
                    # blocks 0+1 share one PSUM bank tile [128, 376]
                    p01 = ps01_pool.tile([128, CPB[0] + CPB[1]], f32, tag="p01")
                    for bi in (0, 1):
                        nc.tensor.matmul(
                            p01[:, COFF[bi] : COFF[bi] + CPB[bi]],
                            lhsT=f1s[:, base + WB[bi] : base + WB[bi] + WN[bi]],
                            rhs=f2s[:, base + US[bi] : base + US[bi] + CPB[bi]],
                            start=True,
                            stop=True,
                        )
                    nc.tensor.matmul(
                        p2[:, dh * CPB[2] : (dh + 1) * CPB[2]],
                        lhsT=f1s[:, base + WB[2] : base + WB[2] + WN[2]],
                        rhs=f2s[:, base + US[2] : base + US[2] + CPB[2]],
                        start=True,
                        stop=True,
                    )
                    # blocks 0+1 -> fp16 staging in one DVE copy
                    nc.vector.tensor_copy(
                        ga[:, h * COFF[2] : (h + 1) * COFF[2]], p01[:]
                    )
                # block-2 of 4 rows -> dense staging in one ACT copy
                nc.scalar.copy(
                    gb[:, h4 * CPB[2] : (h4 + 4) * CPB[2]], p2[0:64, :]
                )
                # dump this 4-row group on the ACT DMA ring
                nc.scalar.dma_start(
                    outa[:, h0 + h4 : h0 + h4 + 4, :],
                    ga[:, h4 * COFF[2] : (h4 + 4) * COFF[2]].rearrange(
                        "p (h c) -> p h c", h=4
                    ),
                )
            nc.scalar.dma_start(
                outb[:, h0 : h0 + hc, :],
                gb[:].rearrange("p (h c) -> p h c", h=hc),
            )

        # software-pipelined emission: chunk k+1's loads+conversions are
        # emitted BEFORE chunk k's compute, so conversions never queue
        # behind PSUM copies on the in-order ACT/DVE streams
        pend = None
        for ci, (h0, hc) in enumerate(chunks):
            tiles = emit_load_conv(ci, h0, hc)
            if pend is not None:
                emit_compute(pend[0], pend[1], *pend[2])
            pend = (h0, hc, tiles)
        emit_compute(pend[0], pend[1], *pend[2])

    nc.finalize()
    return nc


def _run(nc, in_maps, **kwargs):
    from concourse.bass_utils import run_bass_kernel_spmd

    return run_bass_kernel_spmd(nc, in_maps, core_ids=list(range(N_CORES)), **kwargs)


def _assemble(dumps_a, dumps_b):
    """dumps_a: [128, H, 376] fp16 per core (blocks 0+1); dumps_b: [64, H, 104]
    fp16 per core (block 2).

    Block bi covers w = WB[bi]+w'; its dump cols hold G[w, US[bi]+c];
    out[b,j,h,w] = G[w, w+j-40]/C with zeros where w+j-40 is outside [0, W).
    """
    ga = np.stack(dumps_a, axis=0)  # [B, 128, H, 376]
    gbk = np.stack(dumps_b, axis=0)  # [B, 64, H, 104]
    out = np.empty((B, J, H, W), dtype=np.float32)
    z40 = lambda shp: np.zeros(shp, dtype=np.float16)
    for bi in range(3):
        wn = min(WN[bi], W - WB[bi])
        if bi < 2:
            blk = ga[:, :wn, :, COFF[bi] : COFF[bi] + CPB[bi]]
        else:
            blk = gbk
        if bi == 0:
            blk = np.concatenate([z40(blk.shape[:3] + (40,)), blk], axis=3)
        elif bi == 2:
            blk = np.concatenate([blk, z40(blk.shape[:3] + (40,))], axis=3)
        blk = np.ascontiguousarray(blk)
        sb, sw, sh, sc = blk.strides
        band = np.lib.stride_tricks.as_strided(
            blk, shape=(B, H, wn, J), strides=(sb, sh, sw + sc, sc)
        )
        # band[b, h, w', j] -> out[b, j, h, w0+w']
        out[:, :, :, WB[bi] : WB[bi] + wn] = band.transpose(0, 3, 1, 2)
    out *= 1.0 / C
    return out


def kernel(f1: np.ndarray, f2: np.ndarray, **run_kwargs) -> np.ndarray:
    assert f1.shape == (B, C, H, W) and f2.shape == (B, C, H, W)
    nc = _build()
    in_maps = [
        {
            "f1": np.ascontiguousarray(f1[i], dtype=np.float32),
            "f2": np.ascontiguousarray(f2[i], dtype=np.float32),
        }
        for i in range(N_CORES)
    ]
    res = _run(nc, in_maps, **run_kwargs)
    out = _assemble(
        [r["outa"] for r in res.results], [r["outb"] for r in res.results]
    )
    if run_kwargs:
        kernel.last_results = res
    return out


# revision 37
# speedup vs baseline: 1.1737x; 1.1737x over previous
"""Correlation kernel (FlowNet-style, W-displacement only) for Trainium2.

out[b, j, h, w] = mean_c f1[b,c,h,w] * f2pad[b,c,h,w+j],  j in [0, 81), pad=40.

Sharding: data-parallel over batch B=8 across 8 cores (1 batch elem/core).

Device-side work per core (per h row):
  1. Convert f1/f2 chunks fp32 -> fp16 (plain contiguous copies, ACT/DVE).
  2. 3 fp16 matmuls (contraction over C=128 on partitions) produce Gram tiles
     G[w', u] = sum_c f1[c, w0+w'] * f2[c, u0+u] in PSUM (fp32).
  3. ACT/DVE/Pool copy Gram cols PSUM -> SBUF fp16, packed 480 cols/row.
  4. One chunked DMA per 8 rows dumps the packed Gram tiles to DRAM (fp16).

No f2 zero-padding on device: each block's rhs window is clamped to the valid
[0, W) range and the host zero-pads the out-of-range displacements. The
diagonal band extraction (out[j,w] = G[w, w+j-40]) is a shear, which no
on-chip engine can address (per-partition offsets are illegal); the host does
it for free with numpy as_strided during the unshard step. Per-core DRAM
traffic is ~43MB (31.5MB in + 11.8MB out) vs ~84MB for a bounce-based kernel.

DMA queues: input loads ride the SP (sync) ring, dumps ride the ACT (scalar)
ring, so a dump waiting on compute never head-of-line blocks the next chunk's
loads.
"""

import numpy as np
from contextlib import ExitStack

B, C, H, W = 8, 128, 96, 320
D = 40
J = 2 * D + 1  # 81
N_CORES = 8

HCHUNK = 8
NCHUNK = H // HCHUNK
WB = [0, 128, 256]     # w-block starts (lhsT = f1 cols [w0, w0+M))
WN = [128, 128, 128]   # lhsT widths (block 2 spans 64 slack cols)
US = [0, 88, 216]      # rhs window starts (clamped to [0, W))
CPB = [168, 208, 104]  # Gram cols per block (= clamped band cover)
COFF = [0, 168, 376]   # col offsets in the packed dump row
DUMPW = 480
SLACK = 64             # f1h slack so block-2 lhsT can be 128 wide


def _build():
    from concourse import bacc, mybir
    import concourse.tile as tile

    f32 = mybir.dt.float32
    f16 = mybir.dt.float16
    nc = bacc.Bacc(
        "TRN2",
        target_bir_lowering=False,
        debug=False,
        enable_asserts=False,
        num_devices=N_CORES,
    )
    f1 = nc.dram_tensor("f1", [C, H, W], f32, kind="ExternalInput").ap()
    f2 = nc.dram_tensor("f2", [C, H, W], f32, kind="ExternalInput").ap()
    outa = nc.dram_tensor("outa", [128, H, COFF[2]], f16, kind="ExternalOutput").ap()
    outb = nc.dram_tensor("outb", [64, H, CPB[2]], f16, kind="ExternalOutput").ap()

    with tile.TileContext(nc) as tc, ExitStack() as ctx:
        f1r_pool = ctx.enter_context(tc.tile_pool(name="f1r", bufs=3))
        f2r_pool = ctx.enter_context(tc.tile_pool(name="f2r", bufs=3))
        f1h_pool = ctx.enter_context(tc.tile_pool(name="f1h", bufs=2))
        f2h_pool = ctx.enter_context(tc.tile_pool(name="f2h", bufs=2))
        g_pool = ctx.enter_context(tc.tile_pool(name="gsb", bufs=3))
        gb_pool = ctx.enter_context(tc.tile_pool(name="gbsb", bufs=3))
        ps01_pool = ctx.enter_context(tc.tile_pool(name="ps01", bufs=5, space="PSUM"))
        ps2_pool = ctx.enter_context(tc.tile_pool(name="ps2", bufs=3, space="PSUM"))

        chunks = [(i * HCHUNK, HCHUNK) for i in range(NCHUNK - 1)]
        chunks += [((NCHUNK - 1) * HCHUNK, 4), ((NCHUNK - 1) * HCHUNK + 4, 4)]
        for ci, (h0, hc) in enumerate(chunks):
            # fill/drain chunks skip the slow Pool conversion slice so the
            # first matmuls start earlier and the last dumps drain sooner
            edge = ci < 2 or ci >= len(chunks) - 3
            f1r = f1r_pool.tile([C, hc * W], f32)
            nc.sync.dma_start(f1r[:], f1[:, h0 : h0 + hc, :])
            f2r = f2r_pool.tile([C, hc * W], f32)
            nc.sync.dma_start(f2r[:], f2[:, h0 : h0 + hc, :])

            # conversions split across ACT / DVE / Pool by measured rates
            nh = hc * W
            f1s = f1h_pool.tile([C, nh + SLACK], f16)
            nc.scalar.copy(f1s[:, 0:nh], f1r[:, 0:nh])
            nc.gpsimd.memset(f1s[:, nh :], 0.0)
            f2s = f2h_pool.tile([C, nh], f16)
            if edge:
                na = (nh * 9) // 20
                nc.scalar.copy(f2s[:, 0:na], f2r[:, 0:na])
                nc.vector.tensor_copy(f2s[:, na:nh], f2r[:, na:nh])
            else:
                na, nd = nh // 5, nh // 2
                nc.scalar.copy(f2s[:, 0:na], f2r[:, 0:na])
                nc.vector.tensor_copy(f2s[:, na:nd], f2r[:, na:nd])
                nc.gpsimd.tensor_copy(f2s[:, nd:nh], f2r[:, nd:nh])

            ga = g_pool.tile([C, hc * COFF[2]], f16, tag="ga")
            gb = gb_pool.tile([64, hc * CPB[2]], f16, tag="gb")
            for h4 in range(0, hc, 4):
                # block-2 Gram tiles for 4 rows share one PSUM bank
                p2 = ps2_pool.tile([128, 4 * CPB[2]], f32, tag="p2")
                for dh in range(4):
                    h = h4 + dh
                    base = h * W
                    # blocks 0+1 share one PSUM bank tile [128, 376]
                    p01 = ps01_pool.tile([128, CPB[0] + CPB[1]], f32, tag="p01")
                    for bi in (0, 1):
                        nc.tensor.matmul(
                            p01[:, COFF[bi] : COFF[bi] + CPB[bi]],
                            lhsT=f1s[:, base + WB[bi] : base + WB[bi] + WN[bi]],
                            rhs=f2s[:, base + US[bi] : base + US[bi] + CPB[bi]],
                            start=True,
                            stop=True,
                        )
                    nc.tensor.matmul(
                        p2[:, dh * CPB[2] : (dh + 1) * CPB[2]],
                        lhsT=f1s[:, base + WB[2] : base + WB[2] + WN[2]],
                        rhs=f2s[:, base + US[2] : base + US[2] + CPB[2]],
                        start=True,
                        stop=True,
                    )
                    # blocks 0+1 -> fp16 staging in one DVE copy
                    nc.vector.tensor_copy(
                        ga[:, h * COFF[2] : (h + 1) * COFF[2]], p01[:]
                    )
                # block-2 of 4 rows -> dense staging in one ACT copy
                nc.scalar.copy(
                    gb[:, h4 * CPB[2] : (h4 + 4) * CPB[2]], p2[0:64, :]
                )
                # dump this 4-row group on the ACT DMA ring
                nc.scalar.dma_start(
                    outa[:, h0 + h4 : h0 + h4 + 4, :],
                    ga[:, h4 * COFF[2] : (h4 + 4) * COFF[2]].rearrange(
                        "p (h c) -> p h c", h=4
                    ),
                )
            nc.scalar.dma_start(
                outb[:, h0 : h0 + hc, :],
                gb[:].rearrange("p (h c) -> p h c", h=hc),
            )

    nc.finalize()
    return nc


def _run(nc, in_maps, **kwargs):
    from concourse.bass_utils import run_bass_kernel_spmd

    return run_bass_kernel_spmd(nc, in_maps, core_ids=list(range(N_CORES)), **kwargs)


def _assemble(dumps_a, dumps_b):
    """dumps_a: [128, H, 376] fp16 per core (blocks 0+1); dumps_b: [64, H, 104]
    fp16 per core (block 2).

    Block bi covers w = WB[bi]+w'; its dump cols hold G[w, US[bi]+c];
    out[b,j,h,w] = G[w, w+j-40]/C with zeros where w+j-40 is outside [0, W).
    """
    ga = np.stack(dumps_a, axis=0)  # [B, 128, H, 376]
    gbk = np.stack(dumps_b, axis=0)  # [B, 64, H, 104]
    out = np.empty((B, J, H, W), dtype=np.float32)
    z40 = lambda shp: np.zeros(shp, dtype=np.float16)
    for bi in range(3):
        wn = min(WN[bi], W - WB[bi])
        if bi < 2:
            blk = ga[:, :wn, :, COFF[bi] : COFF[bi] + CPB[bi]]
        else:
            blk = gbk
        if bi == 0:
            blk = np.concatenate([z40(blk.shape[:3] + (40,)), blk], axis=3)
        elif bi == 2:
            blk = np.concatenate([blk, z40(blk.shape[:3] + (40,))], axis=3)
        blk = np.ascontiguousarray(blk)
        sb, sw, sh, sc = blk.strides
        band = np.lib.stride_tricks.as_strided(
            blk, shape=(B, H, wn, J), strides=(sb, sh, sw + sc, sc)
        )
        # band[b, h, w', j] -> out[b, j, h, w0+w']
        out[:, :, :, WB[bi] : WB[bi] + wn] = band.transpose(0, 3, 1, 2)
    out *= 1.0 / C
    return out


def kernel(f1: np.ndarray, f2: np.ndarray, **run_kwargs) -> np.ndarray:
    assert f1.shape == (B, C, H, W) and f2.shape == (B, C, H, W)
    nc = _build()
    in_maps = [
        {
            "f1": np.ascontiguousarray(f1[i], dtype=np.float32),
            "f2": np.ascontiguousarray(f2[i], dtype=np.float32),
        }
        for i in range(N_CORES)
    ]
    res = _run(nc, in_maps, **run_kwargs)
    out = _assemble(
        [r["outa"] for r in res.results], [r["outb"] for r in res.results]
    )
    if run_kwargs:
        kernel.last_results = res
    return out


# revision 38
# speedup vs baseline: 1.2294x; 1.0475x over previous
"""Correlation kernel (FlowNet-style, W-displacement only) for Trainium2.

out[b, j, h, w] = mean_c f1[b,c,h,w] * f2pad[b,c,h,w+j],  j in [0, 81), pad=40.

Sharding: data-parallel over batch B=8 across 8 cores (1 batch elem/core).

Device-side work per core (per h row):
  1. Convert f1/f2 chunks fp32 -> fp16 (plain contiguous copies, ACT/DVE).
  2. 3 fp16 matmuls (contraction over C=128 on partitions) produce Gram tiles
     G[w', u] = sum_c f1[c, w0+w'] * f2[c, u0+u] in PSUM (fp32).
  3. ACT/DVE/Pool copy Gram cols PSUM -> SBUF fp16, packed 480 cols/row.
  4. One chunked DMA per 8 rows dumps the packed Gram tiles to DRAM (fp16).

No f2 zero-padding on device: each block's rhs window is clamped to the valid
[0, W) range and the host zero-pads the out-of-range displacements. The
diagonal band extraction (out[j,w] = G[w, w+j-40]) is a shear, which no
on-chip engine can address (per-partition offsets are illegal); the host does
it for free with numpy as_strided during the unshard step. Per-core DRAM
traffic is ~43MB (31.5MB in + 11.8MB out) vs ~84MB for a bounce-based kernel.

DMA queues: input loads ride the SP (sync) ring, dumps ride the ACT (scalar)
ring, so a dump waiting on compute never head-of-line blocks the next chunk's
loads.
"""

import numpy as np
from contextlib import ExitStack

B, C, H, W = 8, 128, 96, 320
D = 40
J = 2 * D + 1  # 81
N_CORES = 8

HCHUNK = 8
NCHUNK = H // HCHUNK
WB = [0, 128, 256]     # w-block starts (lhsT = f1 cols [w0, w0+M))
WN = [128, 128, 128]   # lhsT widths (block 2 spans 64 slack cols)
US = [0, 88, 216]      # rhs window starts (clamped to [0, W))
CPB = [168, 208, 104]  # Gram cols per block (= clamped band cover)
COFF = [0, 168, 376]   # col offsets in the packed dump row
DUMPW = 480
SLACK = 64             # f1h slack so block-2 lhsT can be 128 wide


def _build():
    from concourse import bacc, mybir
    import concourse.tile as tile

    f32 = mybir.dt.float32
    f16 = mybir.dt.float16
    nc = bacc.Bacc(
        "TRN2",
        target_bir_lowering=False,
        debug=False,
        enable_asserts=False,
        num_devices=N_CORES,
    )
    f1 = nc.dram_tensor("f1", [C, H, W], f32, kind="ExternalInput").ap()
    f2 = nc.dram_tensor("f2", [C, H, W], f32, kind="ExternalInput").ap()
    outa = nc.dram_tensor("outa", [128, H, COFF[2]], f16, kind="ExternalOutput").ap()
    outb = nc.dram_tensor("outb", [64, H, CPB[2]], f16, kind="ExternalOutput").ap()

    with tile.TileContext(nc) as tc, ExitStack() as ctx:
        f1r_pool = ctx.enter_context(tc.tile_pool(name="f1r", bufs=3))
        f2r_pool = ctx.enter_context(tc.tile_pool(name="f2r", bufs=3))
        f1h_pool = ctx.enter_context(tc.tile_pool(name="f1h", bufs=2))
        f2h_pool = ctx.enter_context(tc.tile_pool(name="f2h", bufs=2))
        g_pool = ctx.enter_context(tc.tile_pool(name="gsb", bufs=3))
        gb_pool = ctx.enter_context(tc.tile_pool(name="gbsb", bufs=3))
        ps01_pool = ctx.enter_context(tc.tile_pool(name="ps01", bufs=5, space="PSUM"))
        ps2_pool = ctx.enter_context(tc.tile_pool(name="ps2", bufs=3, space="PSUM"))

        chunks = [(0, 4), (4, 4)]
        chunks += [(8 + i * HCHUNK, HCHUNK) for i in range(NCHUNK - 2)]
        chunks += [((NCHUNK - 1) * HCHUNK, 4), ((NCHUNK - 1) * HCHUNK + 4, 4)]
        for ci, (h0, hc) in enumerate(chunks):
            # fill/drain chunks skip the slow Pool conversion slice so the
            # first matmuls start earlier and the last dumps drain sooner
            edge = ci < 2 or ci >= len(chunks) - 3
            f1r = f1r_pool.tile([C, hc * W], f32)
            nc.sync.dma_start(f1r[:], f1[:, h0 : h0 + hc, :])
            f2r = f2r_pool.tile([C, hc * W], f32)
            nc.sync.dma_start(f2r[:], f2[:, h0 : h0 + hc, :])

            # conversions split across ACT / DVE / Pool by measured rates
            nh = hc * W
            f1s = f1h_pool.tile([C, nh + SLACK], f16)
            nc.scalar.copy(f1s[:, 0:nh], f1r[:, 0:nh])
            nc.gpsimd.memset(f1s[:, nh :], 0.0)
            f2s = f2h_pool.tile([C, nh], f16)
            if edge:
                na = (nh * 9) // 20
                nc.scalar.copy(f2s[:, 0:na], f2r[:, 0:na])
                nc.vector.tensor_copy(f2s[:, na:nh], f2r[:, na:nh])
            else:
                na, nd = nh // 5, nh // 2
                nc.scalar.copy(f2s[:, 0:na], f2r[:, 0:na])
                nc.vector.tensor_copy(f2s[:, na:nd], f2r[:, na:nd])
                nc.gpsimd.tensor_copy(f2s[:, nd:nh], f2r[:, nd:nh])

            ga = g_pool.tile([C, hc * COFF[2]], f16, tag="ga")
            gb = gb_pool.tile([64, hc * CPB[2]], f16, tag="gb")
            for h4 in range(0, hc, 4):
                # block-2 Gram tiles for 4 rows share one PSUM bank
                p2 = ps2_pool.tile([128, 4 * CPB[2]], f32, tag="p2")
                for dh in range(4):
                    h = h4 + dh
                    base = h * W
                    # blocks 0+1 share one PSUM bank tile [128, 376]
                    p01 = ps01_pool.tile([128, CPB[0] + CPB[1]], f32, tag="p01")
                    for bi in (0, 1):
                        nc.tensor.matmul(
                            p01[:, COFF[bi] : COFF[bi] + CPB[bi]],
                            lhsT=f1s[:, base + WB[bi] : base + WB[bi] + WN[bi]],
                            rhs=f2s[:, base + US[bi] : base + US[bi] + CPB[bi]],
                            start=True,
                            stop=True,
                        )
                    nc.tensor.matmul(
                        p2[:, dh * CPB[2] : (dh + 1) * CPB[2]],
                        lhsT=f1s[:, base + WB[2] : base + WB[2] + WN[2]],
                        rhs=f2s[:, base + US[2] : base + US[2] + CPB[2]],
                        start=True,
                        stop=True,
                    )
                    # blocks 0+1 -> fp16 staging in one DVE copy
                    nc.vector.tensor_copy(
                        ga[:, h * COFF[2] : (h + 1) * COFF[2]], p01[:]
                    )
                # block-2 of 4 rows -> dense staging in one ACT copy
                nc.scalar.copy(
                    gb[:, h4 * CPB[2] : (h4 + 4) * CPB[2]], p2[0:64, :]
                )
                # dump this 4-row group on the ACT DMA ring
                nc.scalar.dma_start(
                    outa[:, h0 + h4 : h0 + h4 + 4, :],
                    ga[:, h4 * COFF[2] : (h4 + 4) * COFF[2]].rearrange(
                        "p (h c) -> p h c", h=4
                    ),
                )
            nc.scalar.dma_start(
                outb[:, h0 : h0 + hc, :],
                gb[:].rearrange("p (h c) -> p h c", h=hc),
            )

    nc.finalize()
    return nc


def _run(nc, in_maps, **kwargs):
    from concourse.bass_utils import run_bass_kernel_spmd

    return run_bass_kernel_spmd(nc, in_maps, core_ids=list(range(N_CORES)), **kwargs)


def _assemble(dumps_a, dumps_b):
    """dumps_a: [128, H, 376] fp16 per core (blocks 0+1); dumps_b: [64, H, 104]
    fp16 per core (block 2).

    Block bi covers w = WB[bi]+w'; its dump cols hold G[w, US[bi]+c];
    out[b,j,h,w] = G[w, w+j-40]/C with zeros where w+j-40 is outside [0, W).
    """
    ga = np.stack(dumps_a, axis=0)  # [B, 128, H, 376]
    gbk = np.stack(dumps_b, axis=0)  # [B, 64, H, 104]
    out = np.empty((B, J, H, W), dtype=np.float32)
    z40 = lambda shp: np.zeros(shp, dtype=np.float16)
    for bi in range(3):
        wn = min(WN[bi], W - WB[bi])
        if bi < 2:
            blk = ga[:, :wn, :, COFF[bi] : COFF[bi] + CPB[bi]]
        else:
            blk = gbk
        if bi == 0:
            blk = np.concatenate([z40(blk.shape[:3] + (40,)), blk], axis=3)
        elif bi == 2:
            blk = np.concatenate([blk, z40(blk.shape[:3] + (40,))], axis=3)
        blk = np.ascontiguousarray(blk)
        sb, sw, sh, sc = blk.strides
        band = np.lib.stride_tricks.as_strided(
            blk, shape=(B, H, wn, J), strides=(sb, sh, sw + sc, sc)
        )
        # band[b, h, w', j] -> out[b, j, h, w0+w']
        out[:, :, :, WB[bi] : WB[bi] + wn] = band.transpose(0, 3, 1, 2)
    out *= 1.0 / C
    return out


def kernel(f1: np.ndarray, f2: np.ndarray, **run_kwargs) -> np.ndarray:
    assert f1.shape == (B, C, H, W) and f2.shape == (B, C, H, W)
    nc = _build()
    in_maps = [
        {
            "f1": np.ascontiguousarray(f1[i], dtype=np.float32),
            "f2": np.ascontiguousarray(f2[i], dtype=np.float32),
        }
        for i in range(N_CORES)
    ]
    res = _run(nc, in_maps, **run_kwargs)
    out = _assemble(
        [r["outa"] for r in res.results], [r["outb"] for r in res.results]
    )
    if run_kwargs:
        kernel.last_results = res
    return out
